# revision 14
# baseline (speedup 1.0000x reference)
"""DiagonalLSTM Bass/Tile kernel for TRN2 (per-core shard: B=4 images).

DESIGN-E ("plain-p" layout). Key identity: the reference's flat gate split
maps gate q at image-row p, channel k to pre-activation row p' = 16q + (p>>2)
and channel o' = 64*(p%4) + k.  With rhs/PSUM column order col = 4*p' + b,
the matmul output tile P01 (chans o' 0:128) IS the gate tile for positions
p%4 in {0,1} in plain layout: G01[64*c4+k, 64*q + 4*mt + b] = sigmoid(P01)
elementwise, where p = 4*mt + c4.  So:

  - ONE full-tile sigmoid per P tile (2 per step instead of 4 strided ones)
  - gate math is plain 64-col block views
  - the skew-band input copy is a single strided copy per step
  - the unskewed OUT bias-add is a single strided op per step

Per step: 2 A-matmuls (K-packed [Ws1;Ws0] against [h; h_shift]), i2s comes
in as a separate PSUM-init matmul per P tile (off the critical path, rhs is
the per-step skewed input column X), 4 small c2c matmuls on K-packed
[Wc1;Wc0] rhs tiles, 1 upsample matmul (lagged one step).  Gate math is
split across DVE and Pool to shorten the serial chain.  Everything stays
on-chip; DRAM is touched for the initial input load and final output store.
"""
from contextlib import ExitStack

import numpy as np

import concourse.bass as bass
import concourse.tile as tile
from concourse import bacc, mybir

F32 = mybir.dt.float32
BF = mybir.dt.bfloat16
AF = mybir.ActivationFunctionType
ALU = mybir.AluOpType

B = 4          # images per core
H = 64         # rows
W = 64         # cols
C = 64         # input channels
HID = 64       # hidden
NW = H + W - 1 # 127 diagonal steps
DEBUG_MEMSET_OUT = False  # zero OUT first (for truncated-NW debugging)


def v(ap, off, dims):
    """Custom view: keep ap's partition dim, replace free dims, add offset
    (in elements)."""
    return bass.AP(ap.tensor, ap.offset + off, [list(ap.ap[0])] + [list(d) for d in dims])


def dv(ap, off, dims):
    """Fully-custom view (DRAM side of DMAs): absolute offset, all dims."""
    return bass.AP(ap.tensor, off, [[int(s), int(n)] for s, n in dims])


def band(t):
    return max(0, t - (W - 1)), min(H - 1, t)


def build_kernel(ctx, tc, outs, ins):
    nc = tc.nc
    x_d = ins["inputs"]
    out_d = outs["out"]

    def dbg_dump(name, ap, parts, cols):
        """Dump an SBUF tile region to a DRAM debug output, if requested."""
        if name in outs:
            nc.sync.dma_start(
                out=dv(outs[name], 0, [[cols, parts], [1, cols]]),
                in_=ap,
            )

    const = ctx.enter_context(tc.tile_pool(name="const", bufs=1))
    big = ctx.enter_context(tc.tile_pool(name="big", bufs=1))
    st = ctx.enter_context(tc.tile_pool(name="st", bufs=2))
    tmp = ctx.enter_context(tc.tile_pool(name="tmp", bufs=2))
    ps = ctx.enter_context(tc.tile_pool(name="ps", bufs=2, space="PSUM"))

    # ---------------- weights / biases (one-time prep) ----------------
    # lhsT layouts; matmul computes lhsT.T @ rhs.
    # LA01/LA23 [128,128]: rows 0:64 = Ws1[o-blk].T, rows 64:128 = Ws0[o-blk].T
    LA01 = const.tile([128, 128], BF, tag="LA01")
    LA23 = const.tile([128, 128], BF, tag="LA23")
    # LX01/LX23 [64,128]: Wi2s[o-blk].T
    LX01 = const.tile([64, 128], BF, tag="LX01")
    LX23 = const.tile([64, 128], BF, tag="LX23")
    # LC [128,64]: rows 0:64 = Wc1.T, rows 64:128 = Wc0.T
    LC = const.tile([128, 64], BF, tag="LC")
    LU = const.tile([64, 128], BF, tag="LU")      # w_up.T
    LA01f = const.tile([128, 128], F32, tag="LA01f")
    LA23f = const.tile([128, 128], F32, tag="LA23f")
    LX01f = const.tile([64, 128], F32, tag="LX01f")
    LX23f = const.tile([64, 128], F32, tag="LX23f")
    LCf = const.tile([128, 64], F32, tag="LCf")
    LUf = const.tile([64, 128], F32, tag="LUf")
    bt01 = const.tile([128, 2], F32, tag="bt01")
    bt23 = const.tile([128, 2], F32, tag="bt23")
    bsg01 = const.tile([128, 1], F32, tag="bsg01")  # b_i2s + b_s2s, chans 0:128
    bsg23 = const.tile([128, 1], F32, tag="bsg23")
    bc2c2 = const.tile([128, 1], F32, tag="bc2c2")  # b_c2c duplicated
    bup = const.tile([128, 1], F32, tag="bup")

    w_s2s = ins["w_s2s"]   # [256, 64, 2] dram
    w_i2s = ins["w_i2s"]   # [256, 64]
    w_c2c = ins["w_c2c"]   # [64, 64, 2]
    w_up = ins["w_up"]     # [128, 64]

    for blk, (LAf, LXf) in ((0, (LA01f, LX01f)), (1, (LA23f, LX23f))):
        # LA[k, m] = Ws1[128*blk+m, k];  LA[64+k, m] = Ws0[128*blk+m, k]
        nc.sync.dma_start(
            out=LAf[0:64, :],
            in_=dv(w_s2s, 128 * blk * 128 + 1, [[2, 64], [128, 128]]),
        )
        nc.sync.dma_start(
            out=LAf[64:128, :],
            in_=dv(w_s2s, 128 * blk * 128 + 0, [[2, 64], [128, 128]]),
        )
        # LX[c, m] = Wi2s[128*blk+m, c]
        nc.sync.dma_start(
            out=LXf[:, :],
            in_=dv(w_i2s, 128 * blk * 64, [[1, 64], [64, 128]]),
        )
    nc.sync.dma_start(out=LCf[0:64, :], in_=dv(w_c2c, 1, [[2, 64], [128, 64]]))
    nc.sync.dma_start(out=LCf[64:128, :], in_=dv(w_c2c, 0, [[2, 64], [128, 64]]))
    nc.sync.dma_start(out=LUf[:, :], in_=dv(w_up, 0, [[1, 64], [64, 128]]))
    for bf_t, f_t in ((LA01, LA01f), (LA23, LA23f), (LX01, LX01f),
                      (LX23, LX23f), (LC, LCf), (LU, LUf)):
        nc.vector.tensor_copy(bf_t[:, :], f_t[:, :])

    b_i2s, b_s2s, b_c2c, b_up = ins["b_i2s"], ins["b_s2s"], ins["b_c2c"], ins["b_up"]
    for blk, (btile, bout) in ((0, (bt01, bsg01)), (1, (bt23, bsg23))):
        nc.sync.dma_start(out=btile[:, 0:1], in_=dv(b_i2s, 128 * blk, [[1, 128], [1, 1]]))
        nc.sync.dma_start(out=btile[:, 1:2], in_=dv(b_s2s, 128 * blk, [[1, 128], [1, 1]]))
        nc.vector.tensor_add(bout[:, :], btile[:, 0:1], btile[:, 1:2])
    nc.sync.dma_start(out=bc2c2[0:64, :], in_=dv(b_c2c, 0, [[1, 64], [1, 1]]))
    nc.sync.dma_start(out=bc2c2[64:128, :], in_=dv(b_c2c, 0, [[1, 64], [1, 1]]))
    nc.sync.dma_start(out=bup[:, :], in_=dv(b_up, 0, [[1, 128], [1, 1]]))

    # ---------------- input load ----------------
    # IN[c, b*4096 + p*64 + w] = inputs[b, c, p, w]
    IN = big.tile([64, B * H * W], BF, tag="IN")
    for b in range(B):
        nc.sync.dma_start(
            out=IN[:, b * H * W:(b + 1) * H * W],
            in_=dv(x_d, b * C * H * W, [[4096, 64], [1, 4096]]),
        )

    OUT = big.tile([128, B * H * W], F32, tag="OUT")
    IN_ap = IN[:, :]
    OUT_ap = OUT[:, :]
    if DEBUG_MEMSET_OUT:
        for i in range(4):
            nc.gpsimd.memset(OUT[:, i * 4096:(i + 1) * 4096], 0.0)

    # ---------------- helpers ----------------
    def xprep(X, t):
        """Fill X[c, 4p + b] = skewed input col t (zero outside band)."""
        lo, hi = band(t)
        Xap = X[:, :]
        if hi < H - 1:
            nc.gpsimd.memset(v(Xap, 4 * (hi + 1), [[1, 4 * (H - 1 - hi)]]), 0.0)
        if lo > 0:
            nc.gpsimd.memset(v(Xap, 0, [[1, 4 * lo]]), 0.0)
        n = hi - lo + 1
        # src col = 4096*b + 64*p + (t-p) = 4096*b + 63*p + t
        nc.gpsimd.tensor_copy(
            out=v(Xap, 4 * lo, [[4, n], [1, 4]]),
            in_=v(IN_ap, 63 * lo + t, [[63, n], [4096, 4]]),
        )

    # ---------------- initial state ----------------
    A2 = st.tile([128, 256], BF, tag="A2", name="A2_0", bufs=3)
    nc.gpsimd.memset(A2[:, :], 0.0)

    # zero c2c rhs tiles for step 0 (c_{-1} = 0)
    Rp = []
    for i in range(4):
        R = st.tile([128, 64], BF, tag=f"R{i}", name=f"R{i}_0", bufs=2)
        nc.gpsimd.memset(R[:, :], 0.0)
        Rp.append(R)

    # X(0) + x-matmuls(0): PSUM init for step 0
    X = st.tile([64, 256], BF, tag="X", name="X_0", bufs=3)
    xprep(X, 0)
    P01 = ps.tile([128, 256], F32, tag="P01")
    P23 = ps.tile([128, 256], F32, tag="P23")
    nc.tensor.matmul(P01[:, :], LX01[:, :], X[:, :], start=True, stop=False,
                     skip_group_check=True)
    nc.tensor.matmul(P23[:, :], LX23[:, :], X[:, :], start=True, stop=False,
                     skip_group_check=True)

    Uprev = None      # (U tile, t) pending upsample output add
    out_dma_done = 0  # output w-cols already DMA'd

    # ---------------- the recurrence ----------------
    for t in range(NW):
        # --- PE: finish gate pre-activations for step t ---
        nc.tensor.matmul(P01[:, :], LA01[:, :], A2[:, :], start=False, stop=True,
                         skip_group_check=True)
        nc.tensor.matmul(P23[:, :], LA23[:, :], A2[:, :], start=False, stop=True,
                         skip_group_check=True)

        # --- PE: c2c for step t (rhs = R tiles cast last step) ---
        Cp = ps.tile([128, 128], F32, tag="Cp", bufs=1)
        Cp01 = Cp[:, 0:64]
        Cp23 = Cp[:, 64:128]
        nc.tensor.matmul(Cp[0:64, 0:64], LC[:, :], Rp[0][:, :], start=True, stop=True,
                         skip_group_check=True)
        nc.tensor.matmul(Cp[64:128, 0:64], LC[:, :], Rp[1][:, :], start=True, stop=True,
                         skip_group_check=True)
        nc.tensor.matmul(Cp[0:64, 64:128], LC[:, :], Rp[2][:, :], start=True, stop=True,
                         skip_group_check=True)
        nc.tensor.matmul(Cp[64:128, 64:128], LC[:, :], Rp[3][:, :], start=True, stop=True,
                         skip_group_check=True)

        # --- PE: upsample for step t-1 (A2 still holds h_{t-1}) ---
        if t >= 1:
            U = ps.tile([128, 256], F32, tag="U")
            nc.tensor.matmul(U[:, :], LU[:, :], A2[0:64, :], start=True, stop=True)
            Uprev = (U, t - 1)

        # --- ACT: the two sigmoids (plain full tiles) ---
        G01 = tmp.tile([128, 256], F32, tag="G01")
        G23 = tmp.tile([128, 256], F32, tag="G23")
        nc.scalar.activation(G01[:, :], P01[:, :], AF.Sigmoid, bias=bsg01[:, 0:1])
        nc.scalar.activation(G23[:, :], P23[:, :], AF.Sigmoid, bias=bsg23[:, 0:1])

        # --- Pool: x for step t+1, then PE: its PSUM-init matmuls ---
        if t + 1 < NW:
            Xn = st.tile([64, 256], BF, tag="X", name=f"X_{t+1}", bufs=3)
            xprep(Xn, t + 1)
            P01n = ps.tile([128, 256], F32, tag="P01")
            P23n = ps.tile([128, 256], F32, tag="P23")
            nc.tensor.matmul(P01n[:, :], LX01[:, :], Xn[:, :], start=True, stop=False,
                             skip_group_check=True)
            nc.tensor.matmul(P23n[:, :], LX23[:, :], Xn[:, :], start=True, stop=False,
                             skip_group_check=True)

        # --- gate math: chain01 then chain23; DVE/Pool split ---
        A2n = st.tile([128, 256], BF, tag="A2", name=f"A2_{t+1}", bufs=3)
        T2a = tmp.tile([128, 64], F32, tag="T2a")
        T2b = tmp.tile([128, 64], F32, tag="T2b")
        T1a = tmp.tile([128, 64], F32, tag="T1a")
        T1b = tmp.tile([128, 64], F32, tag="T1b")
        C01 = tmp.tile([128, 64], F32, tag="C01")
        C23 = tmp.tile([128, 64], F32, tag="C23")
        TH01 = tmp.tile([128, 64], F32, tag="TH01")
        TH23 = tmp.tile([128, 64], F32, tag="TH23")

        # DVE: T2 = (Cp + b_c2c) * fg ; Pool: T1 = ig * gg
        nc.vector.scalar_tensor_tensor(
            out=T2a[:, :], in0=Cp01, scalar=bc2c2[:, 0:1], in1=G01[:, 128:192],
            op0=ALU.add, op1=ALU.mult,
        )
        nc.gpsimd.tensor_mul(T1a[:, :], G01[:, 0:64], G01[:, 64:128])
        nc.vector.tensor_add(C01[:, :], T1a[:, :], T2a[:, :])
        nc.scalar.activation(TH01[:, :], C01[:, :], AF.Tanh)

        nc.vector.scalar_tensor_tensor(
            out=T2b[:, :], in0=Cp23, scalar=bc2c2[:, 0:1], in1=G23[:, 128:192],
            op0=ALU.add, op1=ALU.mult,
        )
        nc.gpsimd.tensor_mul(T1b[:, :], G23[:, 0:64], G23[:, 64:128])
        nc.vector.tensor_add(C23[:, :], T1b[:, :], T2b[:, :])
        nc.scalar.activation(TH23[:, :], C23[:, :], AF.Tanh)

        # h = og * tanh(c) into A2n lower half (bf16), per c4 class.
        # c4=0 -> cols 16mt+0+b, c4=1 -> +4, c4=2 -> +8, c4=3 -> +12
        A2lo = A2n[0:64, :]
        A2hi = A2n[64:128, :]
        nc.vector.tensor_mul(
            v(A2lo, 0, [[16, 16], [1, 4]]), G01[0:64, 192:256], TH01[0:64, :])
        nc.gpsimd.tensor_mul(
            v(A2lo, 4, [[16, 16], [1, 4]]), G01[64:128, 192:256], TH01[64:128, :])
        nc.vector.tensor_mul(
            v(A2lo, 8, [[16, 16], [1, 4]]), G23[0:64, 192:256], TH23[0:64, :])
        nc.gpsimd.tensor_mul(
            v(A2lo, 12, [[16, 16], [1, 4]]), G23[64:128, 192:256], TH23[64:128, :])

        # h_shift into A2n upper half: col 4p+b <- h[p-1]
        nc.vector.tensor_copy(
            out=v(A2hi, 4, [[16, 16], [1, 4]]), in_=v(A2lo, 0, [[16, 16], [1, 4]]))
        nc.gpsimd.tensor_copy(
            out=v(A2hi, 8, [[16, 16], [1, 4]]), in_=v(A2lo, 4, [[16, 16], [1, 4]]))
        nc.vector.tensor_copy(
            out=v(A2hi, 12, [[16, 16], [1, 4]]), in_=v(A2lo, 8, [[16, 16], [1, 4]]))
        nc.gpsimd.tensor_copy(
            out=v(A2hi, 16, [[16, 15], [1, 4]]), in_=v(A2lo, 12, [[16, 15], [1, 4]]))
        nc.gpsimd.memset(v(A2hi, 0, [[1, 4]]), 0.0)  # h[-1] = 0

        if t == 0:
            dbg_dump("dbg_X", X[:, :], 64, 256)
            dbg_dump("dbg_G01", G01[:, :], 128, 256)
            dbg_dump("dbg_G23", G23[:, :], 128, 256)
            dbg_dump("dbg_C01", C01[:, :], 128, 64)
            dbg_dump("dbg_TH01", TH01[:, :], 128, 64)
            dbg_dump("dbg_A2n", A2n[:, :], 128, 256)

        # --- R casts for step t+1's c2c (c_t taps, bf16, K-packed) ---
        if t + 1 < NW:
            R0 = st.tile([128, 64], BF, tag="R0", name=f"R0_{t+1}", bufs=2)
            R1 = st.tile([128, 64], BF, tag="R1", name=f"R1_{t+1}", bufs=2)
            R2 = st.tile([128, 64], BF, tag="R2", name=f"R2_{t+1}", bufs=2)
            R3 = st.tile([128, 64], BF, tag="R3", name=f"R3_{t+1}", bufs=2)
            # R0: out p=4mt+0: taps c[4mt] (c4=0) & c[4mt-1] (c4=3, mt-1)
            nc.vector.tensor_copy(R1[0:64, :], C01[64:128, :])
            nc.vector.tensor_copy(R1[64:128, :], C01[0:64, :])
            nc.vector.tensor_copy(R3[0:64, :], C23[64:128, :])
            nc.vector.tensor_copy(R3[64:128, :], C23[0:64, :])
            nc.gpsimd.tensor_copy(R0[0:64, :], C01[0:64, :])
            nc.gpsimd.tensor_copy(R0[64:128, 4:64], C23[64:128, 0:60])
            nc.gpsimd.memset(R0[64:128, 0:4], 0.0)
            nc.gpsimd.tensor_copy(R2[0:64, :], C23[0:64, :])
            nc.gpsimd.tensor_copy(R2[64:128, :], C01[64:128, :])
            Rp = [R0, R1, R2, R3]

        # --- OUT += upsample(t-1) + bias (unskew write) ---
        if Uprev is not None:
            Up, tu = Uprev
            lo, hi = band(tu)
            n = hi - lo + 1
            nc.vector.tensor_scalar_add(
                v(OUT_ap, 63 * lo + tu, [[63, n], [4096, 4]]),
                v(Up[:, :], 4 * lo, [[4, n], [1, 4]]),
                bup[:, 0:1],
            )
            Uprev = None

        # --- chunked output store: w-block j final once OUT-add(70+8j) done,
        # which happens at iteration 71+8j (the add lags the step by one) ---
        if t >= 71 and (t - 71) % 8 == 0 and t - 71 < 56:
            j = (t - 71) // 8
            for b in range(B):
                nc.sync.dma_start(
                    out=dv(out_d, b * 128 * H * W + 8 * j,
                           [[4096, 128], [64, 64], [1, 8]]),
                    in_=v(OUT_ap, b * H * W + 8 * j, [[64, 64], [1, 8]]),
                )
            out_dma_done = 8 * (j + 1)

        A2 = A2n
        if t + 1 < NW:
            P01, P23 = P01n, P23n

    # ---------------- tail: last upsample + remaining output ----------------
    U = ps.tile([128, 256], F32, tag="U")
    nc.tensor.matmul(U[:, :], LU[:, :], A2[0:64, :], start=True, stop=True)
    tu = NW - 1
    lo, hi = band(tu)
    n = hi - lo + 1
    nc.vector.tensor_scalar_add(
        v(OUT_ap, 63 * lo + tu, [[63, n], [4096, 4]]),
        v(U[:, :], 4 * lo, [[4, n], [1, 4]]),
        bup[:, 0:1],
    )
    nw_rem = W - out_dma_done
    for b in range(B):
        nc.sync.dma_start(
            out=dv(out_d, b * 128 * H * W + out_dma_done,
                   [[4096, 128], [64, 64], [1, nw_rem]]),
            in_=v(OUT_ap, b * H * W + out_dma_done, [[64, 64], [1, nw_rem]]),
        )


def build_nc():
    nc = bacc.Bacc("TRN2", target_bir_lowering=False, debug=False)
    ins = {
        "inputs": nc.dram_tensor("inputs", [B, C, H, W], BF, kind="ExternalInput").ap(),
        "w_i2s": nc.dram_tensor("w_i2s", [4 * HID, C], F32, kind="ExternalInput").ap(),
        "b_i2s": nc.dram_tensor("b_i2s", [4 * HID], F32, kind="ExternalInput").ap(),
        "w_s2s": nc.dram_tensor("w_s2s", [4 * HID, HID, 2], F32, kind="ExternalInput").ap(),
        "b_s2s": nc.dram_tensor("b_s2s", [4 * HID], F32, kind="ExternalInput").ap(),
        "w_c2c": nc.dram_tensor("w_c2c", [HID, HID, 2], F32, kind="ExternalInput").ap(),
        "b_c2c": nc.dram_tensor("b_c2c", [HID], F32, kind="ExternalInput").ap(),
        "w_up": nc.dram_tensor("w_up", [2 * HID, HID], F32, kind="ExternalInput").ap(),
        "b_up": nc.dram_tensor("b_up", [2 * HID], F32, kind="ExternalInput").ap(),
    }
    outs = {"out": nc.dram_tensor("out", [B, 2 * HID, H, W], F32, kind="ExternalOutput").ap()}
    with tile.TileContext(nc) as tc:
        with ExitStack() as ctx:
            build_kernel(ctx, tc, outs, ins)
    nc.compile()
    return nc


# ---------------------------------------------------------------------------
# Harness entry point: full inputs -> shard over 8 cores -> full output.
# ---------------------------------------------------------------------------
import ml_dtypes
from concourse.bass_utils import run_bass_kernel_spmd

N_CORES = 8
TRACE = False
LAST_EXEC_NS = None
_NC = None


def _get_nc():
    global _NC
    if _NC is None:
        _NC = build_nc()
    return _NC


def kernel(**inputs):
    global LAST_EXEC_NS
    nc = _get_nc()
    full = {k: np.ascontiguousarray(np.asarray(val, np.float32))
            for k, val in inputs.items()}
    xs = full["inputs"].astype(ml_dtypes.bfloat16)
    in_maps = []
    for i in range(N_CORES):
        m = dict(full)
        m["inputs"] = np.ascontiguousarray(xs[B * i:B * (i + 1)])
        in_maps.append(m)
    res = run_bass_kernel_spmd(nc, in_maps, list(range(N_CORES)), trace=TRACE)
    LAST_EXEC_NS = res.exec_time_ns
    return np.concatenate([res.results[i]["out"] for i in range(N_CORES)], axis=0)


# revision 24
# speedup vs baseline: 1.6617x; 1.6617x over previous
"""DiagonalLSTM Bass/Tile kernel for TRN2 (per-core shard: B=4 images).

DESIGN-E ("plain-p" layout). Key identity: the reference's flat gate split
maps gate q at image-row p, channel k to pre-activation row p' = 16q + (p>>2)
and channel o' = 64*(p%4) + k.  With rhs/PSUM column order col = 4*p' + b,
the matmul output tile P01 (chans o' 0:128) IS the gate tile for positions
p%4 in {0,1} in plain layout: G01[64*c4+k, 64*q + 4*mt + b] = sigmoid(P01)
elementwise, where p = 4*mt + c4.  So:

  - ONE full-tile sigmoid per P tile (2 per step instead of 4 strided ones)
  - gate math is plain 64-col block views
  - the skew-band input copy is a single strided copy per step
  - the unskewed OUT bias-add is a single strided op per step

Per step: 2 A-matmuls (K-packed [Ws1;Ws0] against [h; h_shift]), i2s comes
in as a separate PSUM-init matmul per P tile (off the critical path, rhs is
the per-step skewed input column X), 4 small c2c matmuls on K-packed
[Wc1;Wc0] rhs tiles, 1 upsample matmul (lagged one step).  Gate math is
split across DVE and Pool to shorten the serial chain.  Everything stays
on-chip; DRAM is touched for the initial input load and final output store.
"""
from contextlib import ExitStack

import numpy as np

import concourse.bass as bass
import concourse.tile as tile
from concourse import bacc, mybir

F32 = mybir.dt.float32
BF = mybir.dt.bfloat16
AF = mybir.ActivationFunctionType
ALU = mybir.AluOpType

B = 4          # images per core
H = 64         # rows
W = 64         # cols
C = 64         # input channels
HID = 64       # hidden
NW = H + W - 1 # 127 diagonal steps
DEBUG_MEMSET_OUT = False  # zero OUT first (for truncated-NW debugging)
XPREP_ON_ACT = True    # xprep band copy on Scalar engine (else Pool)
AMM_SHIFT = True       # second A-MM with column-shifted rhs view


def v(ap, off, dims):
    """Custom view: keep ap's partition dim, replace free dims, add offset
    (in elements)."""
    return bass.AP(ap.tensor, ap.offset + off, [list(ap.ap[0])] + [list(d) for d in dims])


def dv(ap, off, dims):
    """Fully-custom view (DRAM side of DMAs): absolute offset, all dims."""
    return bass.AP(ap.tensor, off, [[int(s), int(n)] for s, n in dims])


def band(t):
    return max(0, t - (W - 1)), min(H - 1, t)


def build_kernel(ctx, tc, outs, ins):
    nc = tc.nc
    x_d = ins["inputs"]
    out_d = outs["out"]

    def dbg_dump(name, ap, parts, cols):
        """Dump an SBUF tile region to a DRAM debug output, if requested."""
        if name in outs:
            nc.sync.dma_start(
                out=dv(outs[name], 0, [[cols, parts], [1, cols]]),
                in_=ap,
            )

    const = ctx.enter_context(tc.tile_pool(name="const", bufs=1))
    big = ctx.enter_context(tc.tile_pool(name="big", bufs=1))
    st = ctx.enter_context(tc.tile_pool(name="st", bufs=2))
    tmp = ctx.enter_context(tc.tile_pool(name="tmp", bufs=2))
    ps = ctx.enter_context(tc.tile_pool(name="ps", bufs=2, space="PSUM"))

    # ---------------- weights / biases (one-time prep) ----------------
    # lhsT layouts; matmul computes lhsT.T @ rhs.
    # LA01a [64,128] = Ws1[o-blk].T (same-position tap); LA01b = Ws0.T (p-1 tap)
    LA01a = const.tile([64, 128], BF, tag="LA01a")
    LA01b = const.tile([64, 128], BF, tag="LA01b")
    LA23a = const.tile([64, 128], BF, tag="LA23a")
    LA23b = const.tile([64, 128], BF, tag="LA23b")
    # LX01/LX23 [64,128]: Wi2s[o-blk].T
    LX01 = const.tile([64, 128], BF, tag="LX01")
    LX23 = const.tile([64, 128], BF, tag="LX23")
    # c2c lhsT tiles [128,64], zero-padded/K-packed so rhs is always the
    # full base-0 cast tile: LCpk = [Wc0.T; Wc1.T], LCz1 = [Wc1.T; 0],
    # LCz0 = [0; Wc0.T]
    LCpk = const.tile([128, 64], BF, tag="LCpk")
    LCz1 = const.tile([128, 64], BF, tag="LCz1")
    LCz0 = const.tile([128, 64], BF, tag="LCz0")
    LU = const.tile([64, 128], BF, tag="LU")      # w_up.T
    LA01af = const.tile([64, 128], F32, tag="LA01af")
    LA01bf = const.tile([64, 128], F32, tag="LA01bf")
    LA23af = const.tile([64, 128], F32, tag="LA23af")
    LA23bf = const.tile([64, 128], F32, tag="LA23bf")
    LX01f = const.tile([64, 128], F32, tag="LX01f")
    LX23f = const.tile([64, 128], F32, tag="LX23f")
    LCpkf = const.tile([128, 64], F32, tag="LCpkf")
    LCz1f = const.tile([128, 64], F32, tag="LCz1f")
    LCz0f = const.tile([128, 64], F32, tag="LCz0f")
    LUf = const.tile([64, 128], F32, tag="LUf")
    bt01 = const.tile([128, 2], F32, tag="bt01")
    bt23 = const.tile([128, 2], F32, tag="bt23")
    bsg01 = const.tile([128, 1], F32, tag="bsg01")  # b_i2s + b_s2s, chans 0:128
    bsg23 = const.tile([128, 1], F32, tag="bsg23")
    bc2c2 = const.tile([128, 1], F32, tag="bc2c2")  # b_c2c duplicated
    bup = const.tile([128, 1], F32, tag="bup")

    w_s2s = ins["w_s2s"]   # [256, 64, 2] dram
    w_i2s = ins["w_i2s"]   # [256, 64]
    w_c2c = ins["w_c2c"]   # [64, 64, 2]
    w_up = ins["w_up"]     # [128, 64]

    for blk, (LAaf, LAbf, LXf) in ((0, (LA01af, LA01bf, LX01f)),
                                   (1, (LA23af, LA23bf, LX23f))):
        # LAa[k, m] = Ws1[128*blk+m, k];  LAb[k, m] = Ws0[128*blk+m, k]
        nc.sync.dma_start(
            out=LAaf[:, :],
            in_=dv(w_s2s, 128 * blk * 128 + 1, [[2, 64], [128, 128]]),
        )
        nc.sync.dma_start(
            out=LAbf[:, :],
            in_=dv(w_s2s, 128 * blk * 128 + 0, [[2, 64], [128, 128]]),
        )
        # LX[c, m] = Wi2s[128*blk+m, c]
        nc.sync.dma_start(
            out=LXf[:, :],
            in_=dv(w_i2s, 128 * blk * 64, [[1, 64], [64, 128]]),
        )
    nc.sync.dma_start(out=LCpkf[0:64, :], in_=dv(w_c2c, 0, [[2, 64], [128, 64]]))
    nc.sync.dma_start(out=LCpkf[64:128, :], in_=dv(w_c2c, 1, [[2, 64], [128, 64]]))
    nc.sync.dma_start(out=LCz1f[0:64, :], in_=dv(w_c2c, 1, [[2, 64], [128, 64]]))
    nc.gpsimd.memset(LCz1f[64:128, :], 0.0)
    nc.gpsimd.memset(LCz0f[0:64, :], 0.0)
    nc.sync.dma_start(out=LCz0f[64:128, :], in_=dv(w_c2c, 0, [[2, 64], [128, 64]]))
    nc.sync.dma_start(out=LUf[:, :], in_=dv(w_up, 0, [[1, 64], [64, 128]]))
    for bf_t, f_t in ((LA01a, LA01af), (LA01b, LA01bf), (LA23a, LA23af),
                      (LA23b, LA23bf), (LX01, LX01f), (LX23, LX23f),
                      (LCpk, LCpkf), (LCz1, LCz1f), (LCz0, LCz0f), (LU, LUf)):
        nc.vector.tensor_copy(bf_t[:, :], f_t[:, :])

    b_i2s, b_s2s, b_c2c, b_up = ins["b_i2s"], ins["b_s2s"], ins["b_c2c"], ins["b_up"]
    for blk, (btile, bout) in ((0, (bt01, bsg01)), (1, (bt23, bsg23))):
        nc.sync.dma_start(out=btile[:, 0:1], in_=dv(b_i2s, 128 * blk, [[1, 128], [1, 1]]))
        nc.sync.dma_start(out=btile[:, 1:2], in_=dv(b_s2s, 128 * blk, [[1, 128], [1, 1]]))
        nc.vector.tensor_add(bout[:, :], btile[:, 0:1], btile[:, 1:2])
    nc.sync.dma_start(out=bc2c2[0:64, :], in_=dv(b_c2c, 0, [[1, 64], [1, 1]]))
    nc.sync.dma_start(out=bc2c2[64:128, :], in_=dv(b_c2c, 0, [[1, 64], [1, 1]]))
    nc.sync.dma_start(out=bup[:, :], in_=dv(b_up, 0, [[1, 128], [1, 1]]))

    # ---------------- input load ----------------
    # IN[c, b*4096 + p*64 + w] = inputs[b, c, p, w]
    IN = big.tile([64, B * H * W], BF, tag="IN")
    for b in range(B):
        nc.sync.dma_start(
            out=IN[:, b * H * W:(b + 1) * H * W],
            in_=dv(x_d, b * C * H * W, [[4096, 64], [1, 4096]]),
        )

    OUT = big.tile([128, B * H * W], F32, tag="OUT")
    IN_ap = IN[:, :]
    OUT_ap = OUT[:, :]
    if DEBUG_MEMSET_OUT:
        for i in range(4):
            nc.gpsimd.memset(OUT[:, i * 4096:(i + 1) * 4096], 0.0)

    # ---------------- helpers ----------------
    def xprep(X, t):
        """Fill X[c, 4p + b] = skewed input col t (zero outside band)."""
        lo, hi = band(t)
        Xap = X[:, :]
        if hi < H - 1:
            nc.gpsimd.memset(v(Xap, 4 * (hi + 1), [[1, 4 * (H - 1 - hi)]]), 0.0)
        if lo > 0:
            nc.gpsimd.memset(v(Xap, 0, [[1, 4 * lo]]), 0.0)
        n = hi - lo + 1
        # src col = 4096*b + 64*p + (t-p) = 4096*b + 63*p + t
        if XPREP_ON_ACT:
            nc.scalar.activation(
                v(Xap, 4 * lo, [[4, n], [1, 4]]),
                v(IN_ap, 63 * lo + t, [[63, n], [4096, 4]]),
                AF.Copy,
            )
        else:
            nc.gpsimd.tensor_copy(
                out=v(Xap, 4 * lo, [[4, n], [1, 4]]),
                in_=v(IN_ap, 63 * lo + t, [[63, n], [4096, 4]]),
            )

    # ---------------- initial state ----------------
    A2 = st.tile([64, 256], BF, tag="A2", name="A2_0", bufs=3)
    nc.gpsimd.memset(A2[:, :], 0.0)

    # zero c-state casts for step 0 (c_{-1} = 0)
    C01bf = st.tile([128, 64], BF, tag="C01bf", name="C01bf_0", bufs=2)
    C23bf = st.tile([128, 64], BF, tag="C23bf", name="C23bf_0", bufs=2)
    nc.gpsimd.memset(C01bf[:, :], 0.0)
    nc.gpsimd.memset(C23bf[:, :], 0.0)

    # X(0) + x-matmuls(0): PSUM init for step 0
    X = st.tile([64, 256], BF, tag="X", name="X_0", bufs=3)
    xprep(X, 0)
    P01 = ps.tile([128, 256], F32, tag="P01")
    P23 = ps.tile([128, 256], F32, tag="P23")
    nc.tensor.matmul(P01[:, :], LX01[:, :], X[:, :], start=True, stop=False,
                     skip_group_check=True)
    nc.tensor.matmul(P23[:, :], LX23[:, :], X[:, :], start=True, stop=False,
                     skip_group_check=True)

    Uprev = None      # (U tile, t) pending upsample output add
    out_dma_done = 0  # output w-cols already DMA'd

    # ---------------- the recurrence ----------------
    for t in range(NW):
        # --- x for step t+1: Pool memset + ACT strided copy.  Emitted first
        # so the copy occupies the Scalar queue ahead of the sigmoids and runs
        # during the A-matmuls. ---
        if t + 1 < NW:
            Xn = st.tile([64, 256], BF, tag="X", name=f"X_{t+1}", bufs=3)
            xprep(Xn, t + 1)

        # --- PE: finish gate pre-activations for step t ---
        # same-position tap (Ws1) over all cols; p-1 tap (Ws0) via the
        # column-shifted view of the same h tile (out col 4p+b <- h col 4(p-1)+b)
        nc.tensor.matmul(P01[:, :], LA01a[:, :], A2[:, :], start=False, stop=False,
                         skip_group_check=True)
        if AMM_SHIFT:
            nc.tensor.matmul(P01[:, 4:256], LA01b[:, :], A2[:, 0:252], start=False,
                             stop=True, skip_group_check=True)
        nc.tensor.matmul(P23[:, :], LA23a[:, :], A2[:, :], start=False, stop=False,
                         skip_group_check=True)
        if AMM_SHIFT:
            nc.tensor.matmul(P23[:, 4:256], LA23b[:, :], A2[:, 0:252], start=False,
                             stop=True, skip_group_check=True)

        # --- PE: c2c for step t (rhs = plain bf16 casts of c_{t-1}) ---
        Cp = ps.tile([128, 128], F32, tag="Cp", bufs=1)
        Cp01 = Cp[:, 0:64]
        Cp23 = Cp[:, 64:128]
        # All lhsT/rhs at partition base 0 (base-64 operands hang real HW).
        # Zero-padded / K-packed lhsT over the full [c_lo; c_hi] cast tiles:
        #   LCz1 = [Wc1.T; 0], LCz0 = [0; Wc0.T], LCpk = [Wc0.T; Wc1.T].
        # Quadrant taps (out c4 <- Wc1*c[c4] + Wc0*c[c4-1]):
        #   q0: Wc1*c0 + Wc0*c3(mt-1, col-shifted)
        nc.tensor.matmul(Cp[0:64, 0:64], LCz1[:, :], C01bf[:, :],
                         start=True, stop=False, skip_group_check=True)
        nc.tensor.matmul(Cp[0:64, 4:64], LCz0[:, :], C23bf[:, 0:60],
                         start=False, stop=True, skip_group_check=True)
        #   q1: Wc0*c0 + Wc1*c1 -- single K-packed matmul
        nc.tensor.matmul(Cp[64:128, 0:64], LCpk[:, :], C01bf[:, :],
                         start=True, stop=True, skip_group_check=True)
        #   q2: Wc0*c1 + Wc1*c2
        nc.tensor.matmul(Cp[0:64, 64:128], LCz0[:, :], C01bf[:, :],
                         start=True, stop=False, skip_group_check=True)
        nc.tensor.matmul(Cp[0:64, 64:128], LCz1[:, :], C23bf[:, :],
                         start=False, stop=True, skip_group_check=True)
        #   q3: Wc0*c2 + Wc1*c3 -- single K-packed matmul
        nc.tensor.matmul(Cp[64:128, 64:128], LCpk[:, :], C23bf[:, :],
                         start=True, stop=True, skip_group_check=True)

        # --- PE: upsample for step t-1 (A2 still holds h_{t-1}) ---
        if t >= 1:
            U = ps.tile([128, 256], F32, tag="U")
            nc.tensor.matmul(U[:, :], LU[:, :], A2[:, :], start=True, stop=True)
            Uprev = (U, t - 1)

        # --- ACT: the two sigmoids (plain full tiles) ---
        G01 = tmp.tile([128, 256], F32, tag="G01")
        G23 = tmp.tile([128, 256], F32, tag="G23")
        nc.scalar.activation(G01[:, :], P01[:, :], AF.Sigmoid, bias=bsg01[:, 0:1])
        nc.scalar.activation(G23[:, :], P23[:, :], AF.Sigmoid, bias=bsg23[:, 0:1])

        # --- PE: PSUM-init matmuls for step t+1 (queue tail; off-path) ---
        if t + 1 < NW:
            P01n = ps.tile([128, 256], F32, tag="P01")
            P23n = ps.tile([128, 256], F32, tag="P23")
            nc.tensor.matmul(P01n[:, :], LX01[:, :], Xn[:, :], start=True, stop=False,
                             skip_group_check=True)
            nc.tensor.matmul(P23n[:, :], LX23[:, :], Xn[:, :], start=True, stop=False,
                             skip_group_check=True)

        # --- gate math: chain01 then chain23; DVE/Pool split ---
        A2n = st.tile([64, 256], BF, tag="A2", name=f"A2_{t+1}", bufs=3)
        T2a = tmp.tile([128, 64], F32, tag="T2a")
        T2b = tmp.tile([128, 64], F32, tag="T2b")
        T1a = tmp.tile([128, 64], F32, tag="T1a")
        T1b = tmp.tile([128, 64], F32, tag="T1b")
        C01 = tmp.tile([128, 64], F32, tag="C01")
        C23 = tmp.tile([128, 64], F32, tag="C23")
        TH01 = tmp.tile([128, 64], F32, tag="TH01")
        TH23 = tmp.tile([128, 64], F32, tag="TH23")

        # DVE: T2 = (Cp + b_c2c) * fg ; Pool: T1 = ig * gg
        nc.vector.scalar_tensor_tensor(
            out=T2a[:, :], in0=Cp01, scalar=bc2c2[:, 0:1], in1=G01[:, 128:192],
            op0=ALU.add, op1=ALU.mult,
        )
        nc.gpsimd.tensor_mul(T1a[:, :], G01[:, 0:64], G01[:, 64:128])
        nc.vector.tensor_add(C01[:, :], T1a[:, :], T2a[:, :])
        nc.scalar.activation(TH01[:, :], C01[:, :], AF.Tanh)

        nc.vector.scalar_tensor_tensor(
            out=T2b[:, :], in0=Cp23, scalar=bc2c2[:, 0:1], in1=G23[:, 128:192],
            op0=ALU.add, op1=ALU.mult,
        )
        nc.gpsimd.tensor_mul(T1b[:, :], G23[:, 0:64], G23[:, 64:128])
        nc.vector.tensor_add(C23[:, :], T1b[:, :], T2b[:, :])
        nc.scalar.activation(TH23[:, :], C23[:, :], AF.Tanh)

        # h = og * tanh(c) into A2n (bf16), per c4 class.
        # c4=0 -> cols 16mt+0+b, c4=1 -> +4, c4=2 -> +8, c4=3 -> +12
        A2lo = A2n[:, :]
        nc.vector.tensor_mul(
            v(A2lo, 0, [[16, 16], [1, 4]]), G01[0:64, 192:256], TH01[0:64, :])
        nc.gpsimd.tensor_mul(
            v(A2lo, 4, [[16, 16], [1, 4]]), G01[64:128, 192:256], TH01[64:128, :])
        nc.vector.tensor_mul(
            v(A2lo, 8, [[16, 16], [1, 4]]), G23[0:64, 192:256], TH23[0:64, :])
        nc.gpsimd.tensor_mul(
            v(A2lo, 12, [[16, 16], [1, 4]]), G23[64:128, 192:256], TH23[64:128, :])

        if t == 0:
            dbg_dump("dbg_X", X[:, :], 64, 256)
            dbg_dump("dbg_G01", G01[:, :], 128, 256)
            dbg_dump("dbg_G23", G23[:, :], 128, 256)
            dbg_dump("dbg_C01", C01[:, :], 128, 64)
            dbg_dump("dbg_TH01", TH01[:, :], 128, 64)
            dbg_dump("dbg_A2n", A2n[:, :], 64, 256)

        # --- bf16 casts of c_t for step t+1's c2c ---
        if t + 1 < NW:
            C01bf = st.tile([128, 64], BF, tag="C01bf", name=f"C01bf_{t+1}", bufs=2)
            C23bf = st.tile([128, 64], BF, tag="C23bf", name=f"C23bf_{t+1}", bufs=2)
            nc.vector.tensor_copy(C01bf[:, :], C01[:, :])
            nc.vector.tensor_copy(C23bf[:, :], C23[:, :])

        # --- OUT += upsample(t-1) + bias (unskew write) ---
        if Uprev is not None:
            Up, tu = Uprev
            lo, hi = band(tu)
            n = hi - lo + 1
            nc.vector.tensor_scalar_add(
                v(OUT_ap, 63 * lo + tu, [[63, n], [4096, 4]]),
                v(Up[:, :], 4 * lo, [[4, n], [1, 4]]),
                bup[:, 0:1],
            )
            Uprev = None

        # --- chunked output store: image-row block k (rows 8k..8k+8) is final
        # once OUT-add(8k+70) has run, i.e. at iteration 8k+71; rows are
        # contiguous 512-element runs in OUT and in DRAM ---
        if t >= 71 and (t - 71) % 8 == 0 and t - 71 < 56:
            k = (t - 71) // 8
            for b in range(B):
                nc.sync.dma_start(
                    out=dv(out_d, b * 128 * H * W + 512 * k,
                           [[4096, 128], [1, 512]]),
                    in_=OUT[:, b * H * W + 512 * k:b * H * W + 512 * k + 512],
                )
            out_dma_done = 512 * (k + 1)

        A2 = A2n
        if t + 1 < NW:
            P01, P23 = P01n, P23n

    # ---------------- tail: last upsample + remaining output ----------------
    U = ps.tile([128, 256], F32, tag="U")
    nc.tensor.matmul(U[:, :], LU[:, :], A2[:, :], start=True, stop=True)
    tu = NW - 1
    lo, hi = band(tu)
    n = hi - lo + 1
    nc.vector.tensor_scalar_add(
        v(OUT_ap, 63 * lo + tu, [[63, n], [4096, 4]]),
        v(U[:, :], 4 * lo, [[4, n], [1, 4]]),
        bup[:, 0:1],
    )
    rem = H * W - out_dma_done
    for b in range(B):
        nc.sync.dma_start(
            out=dv(out_d, b * 128 * H * W + out_dma_done, [[4096, 128], [1, rem]]),
            in_=OUT[:, b * H * W + out_dma_done:(b + 1) * H * W],
        )


def build_nc():
    nc = bacc.Bacc("TRN2", target_bir_lowering=False, debug=False)
    ins = {
        "inputs": nc.dram_tensor("inputs", [B, C, H, W], BF, kind="ExternalInput").ap(),
        "w_i2s": nc.dram_tensor("w_i2s", [4 * HID, C], F32, kind="ExternalInput").ap(),
        "b_i2s": nc.dram_tensor("b_i2s", [4 * HID], F32, kind="ExternalInput").ap(),
        "w_s2s": nc.dram_tensor("w_s2s", [4 * HID, HID, 2], F32, kind="ExternalInput").ap(),
        "b_s2s": nc.dram_tensor("b_s2s", [4 * HID], F32, kind="ExternalInput").ap(),
        "w_c2c": nc.dram_tensor("w_c2c", [HID, HID, 2], F32, kind="ExternalInput").ap(),
        "b_c2c": nc.dram_tensor("b_c2c", [HID], F32, kind="ExternalInput").ap(),
        "w_up": nc.dram_tensor("w_up", [2 * HID, HID], F32, kind="ExternalInput").ap(),
        "b_up": nc.dram_tensor("b_up", [2 * HID], F32, kind="ExternalInput").ap(),
    }
    outs = {"out": nc.dram_tensor("out", [B, 2 * HID, H, W], F32, kind="ExternalOutput").ap()}
    with tile.TileContext(nc) as tc:
        with ExitStack() as ctx:
            build_kernel(ctx, tc, outs, ins)
    nc.compile()
    return nc


# ---------------------------------------------------------------------------
# Harness entry point: full inputs -> shard over 8 cores -> full output.
# ---------------------------------------------------------------------------
import ml_dtypes
from concourse.bass_utils import run_bass_kernel_spmd

N_CORES = 8
TRACE = False
LAST_EXEC_NS = None
_NC = None


def _get_nc():
    global _NC
    if _NC is None:
        _NC = build_nc()
    return _NC


def kernel(**inputs):
    global LAST_EXEC_NS
    nc = _get_nc()
    full = {k: np.ascontiguousarray(np.asarray(val, np.float32))
            for k, val in inputs.items()}
    xs = full["inputs"].astype(ml_dtypes.bfloat16)
    in_maps = []
    for i in range(N_CORES):
        m = dict(full)
        m["inputs"] = np.ascontiguousarray(xs[B * i:B * (i + 1)])
        in_maps.append(m)
    res = run_bass_kernel_spmd(nc, in_maps, list(range(N_CORES)), trace=TRACE)
    LAST_EXEC_NS = res.exec_time_ns
    return np.concatenate([res.results[i]["out"] for i in range(N_CORES)], axis=0)
